# revision 1
# baseline (speedup 1.0000x reference)
"""Trainium2 Bass kernel for attention pooling (nn_AttentionPooling_26233660244214).

Computation (reference):
    attn = node_feats @ W_attn + b_attn            # [N, 1]
    mask = sigmoid(node_feats @ W_mask + b_mask)   # [N, 1]
    f = attn * mask                                # [N, 1]
    pooled = segment_sum(node_feats * f, batch_idx, 16384)   # [16384, 256]

Strategy: data-parallel over graphs. batch_idx is sorted, so graphs are
contiguous runs of nodes. Each of the 8 cores owns 2048 contiguous graphs,
split into 16 windows of 128 graphs. Host packs each window's nodes into
K fixed-size chunks of 128 nodes (zero-padded), in two layouts concatenated
into ONE fp16 buffer per window (one large DMA per window, ~370 GB/s):
  - node-major  (segment-matmul moving operand)
  - feat-major  (dot-product stationary operand)
On device, per chunk of 128 nodes:
  - TensorE: dots = Xt_chunk.T-contraction against [W_attn|W_mask] -> [128, 2]
  - ScalarE: sig = sigmoid(dots[:,1] + b_mask)
  - VectorE: f = (dots[:,0] + b_attn) * sig
  - VectorE: oh[n, g] = (iota[g] == local_idx[n]) * f[n]         # [128, 128]
  - TensorE: pooled_psum[g, d] += oh.T @ X_chunk  (PSUM accumulate over K)
Window result [128, 256] is copied PSUM->SBUF on ScalarE and DMA'd out on
the Activation HWDGE ring (keeps the SP ring's load stream free of
sem-waits on compute — per-ring DMA execution is FIFO).
Outputs of the 8 cores are concatenated on host (no cross-core reduction:
window boundaries align with graph boundaries).
"""

import os
os.environ.setdefault("JAX_PLATFORMS", "axon,cpu")

import numpy as np
from contextlib import ExitStack

import concourse.bass as bass
import concourse.bacc as bacc
import concourse.tile as tile
from concourse import mybir

N_NODES = 500000
D = 256
G = 16384
NCORES = 8
WIN = 128            # graphs per window
NW = 16              # windows per core
GPC = WIN * NW       # graphs per core

# dtype configuration
DT_X = mybir.dt.float16     # node-major X (segmm rhs) + one-hot lhsT
DT_XT = mybir.dt.float16    # feat-major X (dots lhsT) + W
F32 = mybir.dt.float32

XC_BUFS = 4                 # window-load buffering depth

_prog_cache = {}


def _build_program(nw, k_list, repeat=1):
    """Build the per-core Bass program: window-slot j runs k_list[j] chunks.

    Windows are assigned to (core, slot) by _window_plan so each slot's
    max chunk count over the 8 cores is tight (the SPMD program is shared).
    repeat > 1 wraps the whole computation in a hardware loop executing it
    `repeat` times (for benchmarking: isolates device execution time from
    dispatch/transfer overhead)."""
    k_list = list(k_list)
    kmax = max(k_list)
    k = kmax
    nc = bacc.Bacc("TRN2", target_bir_lowering=False, debug=False)

    # xc = per-window concat of node-major and feat-major layouts: one big
    # 4MB DMA per window (measured 371 GB/s; 8MB two-window granules on
    # alternating rings measured SLOWER — interleaved streams hurt HBM).
    xc = nc.dram_tensor("xc", [nw, 128, 2 * k * 256], DT_X, kind="ExternalInput")
    idxt = nc.dram_tensor("idxt", [128, nw * k], F32, kind="ExternalInput")
    wb = nc.dram_tensor("wb", [128, 4], DT_XT, kind="ExternalInput")
    bb = nc.dram_tensor("bb", [128, 2], F32, kind="ExternalInput")
    # grouped output layout: one 512KB DMA per 4 windows (vs 16 x 128KB);
    # host unscatters (it already permutes windows anyway)
    out = nc.dram_tensor("out", [nw // 4, 128, 4 * 256], F32, kind="ExternalOutput")

    with tile.TileContext(nc) as tc, ExitStack() as ctx:
        const_pool = ctx.enter_context(tc.tile_pool(name="const", bufs=1))
        xc_pool = ctx.enter_context(tc.tile_pool(name="xc", bufs=XC_BUFS))
        dots_pool = ctx.enter_context(tc.tile_pool(name="dots", bufs=6, space="PSUM"))
        pool_psum = ctx.enter_context(tc.tile_pool(name="pool", bufs=2, space="PSUM"))
        small_pool = ctx.enter_context(tc.tile_pool(name="small", bufs=12))
        oh_pool = ctx.enter_context(tc.tile_pool(name="oh", bufs=8))
        out_pool = ctx.enter_context(tc.tile_pool(name="outp", bufs=2))
        assert nw % 4 == 0

        # constants
        iota_i = const_pool.tile([128, 128], mybir.dt.int32)
        nc.gpsimd.iota(iota_i[:], pattern=[[1, 128]], base=0, channel_multiplier=0)
        iota_f = const_pool.tile([128, 128], DT_X)
        nc.vector.tensor_copy(iota_f[:], iota_i[:])
        idx_sb = const_pool.tile([128, nw * k], F32)
        nc.sync.dma_start(idx_sb[:], idxt.ap())
        wb_sb = const_pool.tile([128, 4], DT_XT)
        nc.sync.dma_start(wb_sb[:], wb.ap())
        bb_sb = const_pool.tile([128, 2], F32)
        nc.sync.dma_start(bb_sb[:], bb.ap())

        out_ap = out.ap()

        def body(_iv=None):
            _emit_windows(nc, tc, nw, k, k_list, xc, out_ap, iota_f, idx_sb,
                          wb_sb, bb_sb, xc_pool, dots_pool,
                          pool_psum, small_pool, oh_pool, out_pool)

        if repeat > 1:
            with tc.For_i(0, repeat, 1):
                body()
        else:
            body()

    nc.compile()
    return nc


def _emit_windows(nc, tc, nw, k, k_list, xc, out_ap, iota_f, idx_sb, wb_sb,
                  bb_sb, xc_pool, dots_pool, pool_psum, small_pool,
                  oh_pool, out_pool):
    sigmoid = mybir.ActivationFunctionType.Sigmoid
    alu = mybir.AluOpType
    out4 = None
    for w in range(nw):
        kw = k_list[w]
        xc_t = xc_pool.tile([128, 2 * k * 256], DT_X)
        # transfer only the used prefix (per-slot chunk count)
        nc.sync.dma_start(xc_t[:, 0 : 2 * kw * 256],
                          xc.ap()[w][:, 0 : 2 * kw * 256])
        xn_t = xc_t[:, 0 : kw * 256]
        xt_t = xc_t[:, kw * 256 : 2 * kw * 256]

        pool_ps = pool_psum.tile([128, 256], F32)
        for c in range(kw):
            j = w * k + c
            # dots[node, 0:2] = sum_feat X[node, feat] * [W_attn | W_mask]
            dots_ps = dots_pool.tile([128, 2], F32)
            nc.tensor.matmul(
                dots_ps[:], lhsT=xt_t[:, c * 256 : c * 256 + 128],
                rhs=wb_sb[:, 0:2], start=True, stop=False,
            )
            nc.tensor.matmul(
                dots_ps[:], lhsT=xt_t[:, c * 256 + 128 : c * 256 + 256],
                rhs=wb_sb[:, 2:4], start=False, stop=True,
            )
            # sig = sigmoid(dots[:,1] + b_mask)
            sig = small_pool.tile([128, 1], F32, tag="sig")
            nc.scalar.activation(sig[:], dots_ps[:, 1:2], sigmoid,
                                 bias=bb_sb[:, 1:2], scale=1.0)
            # f = (dots[:,0] + b_attn) * sig
            f_t = small_pool.tile([128, 1], F32, tag="f")
            nc.vector.scalar_tensor_tensor(
                f_t[:], in0=dots_ps[:, 0:1], scalar=bb_sb[:, 0:1],
                in1=sig[:], op0=alu.add, op1=alu.mult,
            )
            # oh[n, g] = (iota[g] == idx[n]) * f[n]
            oh = oh_pool.tile([128, 128], DT_X)
            nc.vector.tensor_scalar(
                out=oh[:], in0=iota_f[:], scalar1=idx_sb[:, j : j + 1],
                scalar2=f_t[:], op0=alu.is_equal, op1=alu.mult,
            )
            # pooled[g, d] += sum_n oh[n, g] * X[n, d]
            nc.tensor.matmul(
                pool_ps[:], lhsT=oh[:], rhs=xn_t[:, c * 256 : (c + 1) * 256],
                start=(c == 0), stop=(c == kw - 1),
            )
        if w % 4 == 0:
            out4 = out_pool.tile([128, 4 * 256], F32, tag="out4")
        nc.scalar.copy(out4[:, (w % 4) * 256 : (w % 4 + 1) * 256], pool_ps[:])
        if w % 4 == 3:
            # out DMA on the Activation HWDGE ring: keeps the SP ring's load
            # stream free of sem-waits on compute (FIFO per-ring stalls).
            nc.scalar.dma_start(out_ap[w // 4], out4[:])


def _window_plan(batch_idx, nw):
    """Assign the NCORES*nw global windows to (core, slot) so each slot's
    max chunk count over cores is tight, and return (perm, k_list).
    perm[core][slot] = global window id; k_list[slot] = chunk count."""
    bi = np.asarray(batch_idx, dtype=np.int64)
    win_graphs = G // (NCORES * nw)
    bounds = np.searchsorted(bi, np.arange(0, G + 1, win_graphs))
    counts = np.diff(bounds)
    k_all = np.maximum(2, np.ceil(counts / 128).astype(int))
    order = np.argsort(-k_all, kind="stable")
    perm = [[0] * nw for _ in range(NCORES)]
    k_list = []
    for j in range(nw):
        grp = order[j * NCORES : (j + 1) * NCORES]
        k_list.append(int(k_all[grp].max()))
        for c in range(NCORES):
            perm[c][j] = int(grp[c])
    return perm, tuple(k_list), bounds


def _pack_inputs(node_feats, batch_idx, W_attn, b_attn, W_mask, b_mask, nw,
                 perm, k_list, bounds):
    """Pack full inputs into per-core input maps."""
    nf = np.ascontiguousarray(np.asarray(node_feats, dtype=np.float32))
    bi = np.asarray(batch_idx, dtype=np.int64)
    win_graphs = G // (NCORES * nw)
    k = max(k_list)

    np_x = mybir.dt.np(DT_X)
    np_xt = mybir.dt.np(DT_XT)
    in_maps = []
    for core in range(NCORES):
        xcb = np.zeros((nw, 128, 2 * k * 256), dtype=np_x)
        idxt = np.full((128, nw * k), -1.0, dtype=np.float32)
        for w in range(nw):
            gw = perm[core][w]
            kw = k_list[w]
            s, e = int(bounds[gw]), int(bounds[gw + 1])
            n = e - s
            buf = np.zeros((kw * 128, 256), dtype=np.float32)
            buf[:n] = nf[s:e]
            b3 = buf.reshape(kw, 128, 256)
            # node-major: [p, c*256 + d] = buf[c*128+p, d]
            xcb[w, :, 0 : kw * 256] = (
                b3.transpose(1, 0, 2).reshape(128, kw * 256).astype(np_x))
            # feat-major: [p, c*256 + h*128 + nn] = buf[c*128+nn, h*128+p]
            b4 = b3.reshape(kw, 128, 2, 128)  # [c, nn, h, p]
            xcb[w, :, kw * 256 : 2 * kw * 256] = (
                b4.transpose(3, 0, 2, 1).reshape(128, kw * 256).astype(np_xt))
            # local graph index per node: [p, w*k + c] = idx[c*128+p] - gw*win
            ib = np.full((kw * 128,), -1.0, dtype=np.float32)
            ib[:n] = (bi[s:e] - gw * win_graphs).astype(np.float32)
            idxt[:, w * k : w * k + kw] = ib.reshape(kw, 128).T
        wbv = np.zeros((128, 4), dtype=np.float32)
        wa = np.asarray(W_attn, dtype=np.float32).reshape(256)
        wm = np.asarray(W_mask, dtype=np.float32).reshape(256)
        wbv[:, 0] = wa[0:128]
        wbv[:, 1] = wm[0:128]
        wbv[:, 2] = wa[128:256]
        wbv[:, 3] = wm[128:256]
        bbv = np.zeros((128, 2), dtype=np.float32)
        bbv[:, 0] = np.float32(np.asarray(b_attn).reshape(-1)[0])
        bbv[:, 1] = np.float32(np.asarray(b_mask).reshape(-1)[0])
        in_maps.append({
            "xc": xcb, "idxt": idxt,
            "wb": wbv.astype(np_xt), "bb": bbv,
        })
    return in_maps


def _compute_k(batch_idx, nw):
    bi = np.asarray(batch_idx, dtype=np.int64)
    win_graphs = G // (NCORES * nw)
    bounds = np.searchsorted(bi, np.arange(0, G + 1, win_graphs))
    counts = np.diff(bounds)
    return max(2, int(np.ceil(counts.max() / 128)))


class _Runner:
    """Compiled SPMD executable with device-resident input support."""

    def __init__(self, nc, n_cores):
        import jax
        from jax.sharding import Mesh, PartitionSpec
        from jax.experimental.shard_map import shard_map
        from concourse.bass2jax import _bass_exec_p, install_neuronx_cc_hook, \
            partition_id_tensor

        install_neuronx_cc_hook()
        in_names, out_names, out_avals, zero_outs = [], [], [], []
        partition_name = (nc.partition_id_tensor.name
                          if nc.partition_id_tensor else None)
        for alloc in nc.m.functions[0].allocations:
            if not isinstance(alloc, mybir.MemoryLocationSet):
                continue
            name = alloc.memorylocations[0].name
            if alloc.kind == "ExternalInput":
                if name != partition_name:
                    in_names.append(name)
            elif alloc.kind == "ExternalOutput":
                shape = tuple(alloc.tensor_shape)
                dtype = mybir.dt.np(alloc.dtype)
                out_names.append(name)
                out_avals.append(jax.core.ShapedArray(shape, dtype))
                zero_outs.append(np.zeros(shape, dtype))
        self.n_params = len(in_names)
        self.in_names = list(in_names)
        self.out_names = out_names
        all_names = in_names + out_names
        if partition_name is not None:
            all_names.append(partition_name)

        def _body(*args):
            operands = list(args)
            if partition_name is not None:
                operands.append(partition_id_tensor())
            outs = _bass_exec_p.bind(
                *operands,
                out_avals=tuple(out_avals),
                in_names=tuple(all_names),
                out_names=tuple(out_names),
                lowering_input_output_aliases=(),
                sim_require_finite=True,
                sim_require_nnan=True,
                nc=nc,
            )
            return tuple(outs)

        devices = jax.devices()[:n_cores]
        self.mesh = Mesh(np.asarray(devices), ("core",))
        n_in = self.n_params + len(out_names)
        self.jitted = jax.jit(
            shard_map(_body, mesh=self.mesh,
                      in_specs=(PartitionSpec("core"),) * n_in,
                      out_specs=(PartitionSpec("core"),) * len(out_names),
                      check_rep=False),
            keep_unused=True,
        )
        self.zero_outs = zero_outs
        self.n_cores = n_cores
        self._jax = jax
        self._P = PartitionSpec

    def put_inputs(self, in_maps):
        """Concatenate per-core inputs and place on device."""
        import jax
        from jax.sharding import NamedSharding
        arrs = []
        for i, name in enumerate(self.in_names):
            cat = np.concatenate([np.asarray(m[name]) for m in in_maps], axis=0)
            arrs.append(cat)
        for z in self.zero_outs:
            arrs.append(np.concatenate([z] * self.n_cores, axis=0))
        sh = NamedSharding(self.mesh, self._P("core"))
        return [jax.device_put(a, sh) for a in arrs]

    def run(self, dev_args):
        return self.jitted(*dev_args)


_runner_cache = {}


def _get_runner(nw, k_list):
    key = (nw, k_list)
    if key not in _runner_cache:
        if key not in _prog_cache:
            _prog_cache[key] = _build_program(nw, k_list)
        _runner_cache[key] = _Runner(_prog_cache[key], NCORES)
    return _runner_cache[key]


def kernel(node_feats, batch_idx, W_attn, b_attn, W_mask, b_mask):
    from concourse.bass_utils import run_bass_kernel_spmd
    nw = NW
    perm, k_list, bounds = _window_plan(batch_idx, nw)
    key = (nw, k_list)
    if key not in _prog_cache:
        _prog_cache[key] = _build_program(nw, k_list)
    nc = _prog_cache[key]
    in_maps = _pack_inputs(node_feats, batch_idx, W_attn, b_attn, W_mask,
                           b_mask, nw, perm, k_list, bounds)
    res = run_bass_kernel_spmd(nc, in_maps, list(range(NCORES)))
    final = np.zeros((G, 256), dtype=np.float32)
    for c in range(NCORES):
        o = np.asarray(res.results[c]["out"], dtype=np.float32)
        for j in range(nw):
            gw = perm[c][j]
            final[gw * WIN : (gw + 1) * WIN] = (
                o[j // 4, :, (j % 4) * 256 : (j % 4 + 1) * 256])
    return final


def _bench_calls(nw, k_list, repeat, in_maps, n_calls=10, warmup=2):
    """Sequential blocking calls of the repeat-looped program; returns list
    of per-call wall times (device execution repeats the computation
    `repeat` times inside one NEFF dispatch)."""
    import time
    key = (nw, k_list, repeat)
    if key not in _runner_cache:
        _runner_cache[key] = _Runner(_build_program(nw, k_list, repeat=repeat),
                                     NCORES)
    runner = _runner_cache[key]
    dev_args = runner.put_inputs(in_maps)
    times = []
    for i in range(warmup + n_calls):
        t0 = time.perf_counter()
        r = runner.run(dev_args)
        np.asarray(r[0])  # force d2h fetch => true completion
        dt = time.perf_counter() - t0
        if i >= warmup:
            times.append(dt)
    return times


def benchmark(node_feats, batch_idx, W_attn, b_attn, W_mask, b_mask,
              r_small=1, r_big=2049):
    """Estimate per-execution device time in ns via repeat-loop differencing."""
    nw = NW
    perm, k_list, bounds = _window_plan(batch_idx, nw)
    in_maps = _pack_inputs(node_feats, batch_idx, W_attn, b_attn, W_mask,
                           b_mask, nw, perm, k_list, bounds)
    t1 = _bench_calls(nw, k_list, r_small, in_maps)
    t2 = _bench_calls(nw, k_list, r_big, in_maps)
    per_exec = (min(t2) - min(t1)) / (r_big - r_small)
    return per_exec * 1e9, min(t1), min(t2), t1, t2



# revision 16
# speedup vs baseline: 1.5703x; 1.5703x over previous
"""Trainium2 Bass kernel for attention pooling (nn_AttentionPooling_26233660244214).

Computation (reference):
    attn = node_feats @ W_attn + b_attn            # [N, 1]
    mask = sigmoid(node_feats @ W_mask + b_mask)   # [N, 1]
    f = attn * mask                                # [N, 1]
    pooled = segment_sum(node_feats * f, batch_idx, 16384)   # [16384, 256]

Strategy: data-parallel over graphs. batch_idx is sorted, so graphs are
contiguous runs of nodes. Each of the 8 cores owns 2048 contiguous graphs,
split into 16 windows of 128 graphs. Host packs each window's nodes into
K fixed-size chunks of 128 nodes (zero-padded) in ONE node-major fp16
layout (single copy of X -> half the HBM traffic of a two-layout scheme).
On device, per chunk of 128 nodes:
  - TensorE transpose (fp32 view = packed fp16 feat pairs) of the chunk
    into PSUM: one [128,128]xf32 is_transpose matmul per chunk. This is an
    exact bit-movement; the fp32 element (X[n,2j], X[n,2j+1]) lands
    transposed so the fp16 view of the output holds X[n,2j] at [j,2n] and
    X[n,2j+1] at [j,2n+1].
  - Act/DVE (alternating per 4-chunk group): copy the transposed group
    PSUM->SBUF ([128, 4*128] fp32, one instruction amortizes access
    latency).
  - TensorE: dots[n,0:2] via 2 matmuls with stride-2 fp16 stationary
    slices (even feats / odd feats) against interleaved [W_attn|W_mask].
  - ScalarE: sig = sigmoid(dots[:,1] + b_mask)
  - VectorE: f = (dots[:,0] + b_attn) * sig
  - VectorE: oh[n, g] = (iota[g] == local_idx[n]) * f[n]         # [128, 128]
  - TensorE: pooled_psum[g, d] += oh.T @ X_chunk  (PSUM accumulate over K)
Transposes are software-pipelined one group ahead of the dots/pool tail so
the PE keeps busy while the copy engine drains the previous group.
Window result [128, 256] is copied PSUM->SBUF on ScalarE and DMA'd out on
the Activation HWDGE ring. Outputs of the 8 cores are concatenated on host
(no cross-core reduction: window boundaries align with graph boundaries).
"""

import os
os.environ.setdefault("JAX_PLATFORMS", "axon,cpu")

import numpy as np
from contextlib import ExitStack

import concourse.bass as bass
import concourse.bacc as bacc
import concourse.tile as tile
from concourse import mybir

N_NODES = 500000
D = 256
G = 16384
NCORES = 8
WIN = 128            # graphs per window
NW = 16              # windows per core
GPC = WIN * NW       # graphs per core
TGRP = 4             # chunks per transpose/copy group

# dtype configuration
DT_X = mybir.dt.float16     # node-major X (pooled rhs; fp32-paired for transpose)
F32 = mybir.dt.float32

XC_BUFS = int(os.environ.get('K_XCBUFS', '4'))  # window-load buffering depth

_prog_cache = {}


def _build_program(nw, k_list, repeat=1):
    """Build the per-core Bass program: window-slot j runs k_list[j] chunks.

    Windows are assigned to (core, slot) by _window_plan so each slot's
    max chunk count over the 8 cores is tight (the SPMD program is shared).
    repeat > 1 wraps the whole computation in a hardware loop executing it
    `repeat` times (for benchmarking)."""
    k_list = list(k_list)
    k = max(k_list)
    nc = bacc.Bacc("TRN2", target_bir_lowering=False, debug=False)

    # xc = node-major fp16 per window: one DMA per window (~370 GB/s).
    xc = nc.dram_tensor("xc", [nw, 128, k * 256], DT_X, kind="ExternalInput")
    idxt = nc.dram_tensor("idxt", [128, nw * k], F32, kind="ExternalInput")
    wb = nc.dram_tensor("wb", [128, 4], DT_X, kind="ExternalInput")
    bb = nc.dram_tensor("bb", [128, 2], F32, kind="ExternalInput")
    # grouped output layout: one 512KB DMA per 4 windows (vs 16 x 128KB);
    # host unscatters (it already permutes windows anyway)
    out = nc.dram_tensor("out", [nw // 2, 128, 2 * 256], F32, kind="ExternalOutput")

    with tile.TileContext(nc) as tc, ExitStack() as ctx:
        const_pool = ctx.enter_context(tc.tile_pool(name="const", bufs=1))
        xc_pool = ctx.enter_context(tc.tile_pool(name="xc", bufs=XC_BUFS))
        tp_pool = ctx.enter_context(tc.tile_pool(name="tp", bufs=2, space="PSUM"))
        xt_pool = ctx.enter_context(tc.tile_pool(name="xt", bufs=4))
        dots_pool = ctx.enter_context(tc.tile_pool(name="dots", bufs=int(os.environ.get("K_DOTBUFS","4")), space="PSUM"))
        pool_psum = ctx.enter_context(tc.tile_pool(name="pool", bufs=int(os.environ.get("K_PLBUFS","2")), space="PSUM"))
        small_pool = ctx.enter_context(tc.tile_pool(name="small", bufs=12))
        oh_pool = ctx.enter_context(tc.tile_pool(name="oh", bufs=int(os.environ.get("K_OHBUFS","12"))))
        out_pool = ctx.enter_context(tc.tile_pool(name="outp", bufs=int(os.environ.get("K_OUTBUFS","2"))))
        assert nw % 2 == 0

        # constants
        iota_i = const_pool.tile([128, 128], mybir.dt.int32)
        nc.gpsimd.iota(iota_i[:], pattern=[[1, 128]], base=0, channel_multiplier=0)
        iota_f = const_pool.tile([128, 128], DT_X)
        nc.vector.tensor_copy(iota_f[:], iota_i[:])
        # identity (fp32) for TensorE transposes: ident[p, f] = (f == p)
        iota_ci = const_pool.tile([128, 1], mybir.dt.int32)
        nc.gpsimd.iota(iota_ci[:], pattern=[[1, 1]], base=0, channel_multiplier=1)
        iota_cf = const_pool.tile([128, 1], F32)
        nc.vector.tensor_copy(iota_cf[:], iota_ci[:])
        iota_rf = const_pool.tile([128, 128], F32)
        nc.vector.tensor_copy(iota_rf[:], iota_i[:])
        ident32 = const_pool.tile([128, 128], F32)
        nc.vector.tensor_scalar(
            out=ident32[:], in0=iota_rf[:], scalar1=iota_cf[:], scalar2=None,
            op0=mybir.AluOpType.is_equal,
        )
        idx_sb = const_pool.tile([128, nw * k], F32)
        wb_sb = const_pool.tile([128, 4], DT_X)
        bb_sb = const_pool.tile([128, 2], F32)
        const_dmas = [(wb_sb, wb), (bb_sb, bb), (idx_sb, idxt)]

        out_ap = out.ap()

        def body(_iv=None):
            _emit_windows(nc, tc, nw, k, k_list, xc, out_ap, iota_f, ident32,
                          idx_sb, wb_sb, bb_sb, xc_pool, tp_pool, xt_pool,
                          dots_pool, pool_psum, small_pool, oh_pool, out_pool,
                          const_dmas)

        if repeat > 1:
            with tc.For_i(0, repeat, 1):
                body()
        else:
            body()

    nc.compile()
    return nc


def _emit_windows(nc, tc, nw, k, k_list, xc, out_ap, iota_f, ident32, idx_sb,
                  wb_sb, bb_sb, xc_pool, tp_pool, xt_pool, dots_pool,
                  pool_psum, small_pool, oh_pool, out_pool, const_dmas=None):
    """Three-stage software pipeline over 4-chunk groups:

      cycle g emits, in this order:
        PE : transposes(g)                  [tp_ps(g) <- X(g)]
        PE : dots(g-1)                      [reads xt_sb(g-1)]
        Act: sig(g-1); DVE: f(g-1), oh(g-1)
        Act/DVE (alternating): copy(g)      [xt_sb(g) <- tp_ps(g)]
        PE : pooled(g-2) (+ window-close output copy/DMA)

    The copy is emitted AFTER the g-1 small ops so Act's queue runs the
    sigmoids before the long copy (no head-of-line blocking), and pooled
    runs a full cycle after oh so the PE never stalls on the one-hot."""
    sigmoid = mybir.ActivationFunctionType.Sigmoid
    alu = mybir.AluOpType

    state = {"out4": None, "copy_tog": 0}

    def make_stage1(xc_t, xt_sb, w, kw, g0, gc, pool_ps):
        oh_list = []

        def stage1():
            xt16 = xt_sb[:].bitcast(DT_X)  # [128, TGRP*256]
            dots_list = []
            for ci in range(gc):
                base = ci * 256
                # dots[n,0:2] = sum_j X[n,2j]*W[2j] + sum_j X[n,2j+1]*W[2j+1]
                dots_ps = dots_pool.tile([128, 2], F32)
                nc.tensor.matmul(
                    dots_ps[:], lhsT=xt16[:, base : base + 256 : 2],
                    rhs=wb_sb[:, 0:2], start=True, stop=False,
                )
                nc.tensor.matmul(
                    dots_ps[:], lhsT=xt16[:, base + 1 : base + 256 : 2],
                    rhs=wb_sb[:, 2:4], start=False, stop=True,
                )
                dots_list.append(dots_ps)
            sig_list = []
            for ci in range(gc):
                sig = small_pool.tile([128, 1], F32, tag="sig")
                nc.scalar.activation(sig[:], dots_list[ci][:, 1:2], sigmoid,
                                     bias=bb_sb[:, 1:2], scale=1.0)
                sig_list.append(sig)
            for ci in range(gc):
                c = g0 + ci
                j = w * k + c
                f_t = small_pool.tile([128, 1], F32, tag="f")
                nc.vector.scalar_tensor_tensor(
                    f_t[:], in0=dots_list[ci][:, 0:1], scalar=bb_sb[:, 0:1],
                    in1=sig_list[ci][:], op0=alu.add, op1=alu.mult,
                )
                oh = oh_pool.tile([128, 128], DT_X)
                nc.vector.tensor_scalar(
                    out=oh[:], in0=iota_f[:], scalar1=idx_sb[:, j : j + 1],
                    scalar2=f_t[:], op0=alu.is_equal, op1=alu.mult,
                )
                oh_list.append(oh)

        def stage2():
            for ci in range(gc):
                c = g0 + ci
                nc.tensor.matmul(
                    pool_ps[:], lhsT=oh_list[ci][:],
                    rhs=xc_t[:, c * 256 : (c + 1) * 256],
                    start=(c == 0), stop=(c == kw - 1),
                )
            if g0 + gc == kw:
                # window closed: copy result out of PSUM, DMA per 4 windows
                if w % 2 == 0:
                    state["out4"] = out_pool.tile([128, 2 * 256], F32,
                                                  name="out4", tag="out4")
                out4 = state["out4"]
                if os.environ.get('K_OUTDVE', '1') == '1':
                    nc.vector.tensor_copy(
                        out4[:, (w % 2) * 256 : (w % 2 + 1) * 256], pool_ps[:])
                else:
                    nc.scalar.copy(
                        out4[:, (w % 2) * 256 : (w % 2 + 1) * 256], pool_ps[:])
                if w % 2 == 1:
                    if os.environ.get('K_OUTPOOL', '1') == '1':
                        nc.gpsimd.dma_start(out_ap[w // 2], out4[:])
                    else:
                        nc.scalar.dma_start(out_ap[w // 2], out4[:])

        return stage1, stage2

    ST1_LAG = 2       # dots/sig/f/oh run 2 cycles after the transpose
    ST2_LAG = int(os.environ.get('K_ST2LAG', '4'))
    st1_q = []        # pending stage1 closures
    st2_q = []        # pending stage2 closures
    for w in range(nw):
        kw = k_list[w]
        xc_t = xc_pool.tile([128, k * 256], DT_X)
        if w <= int(os.environ.get('K_SPLITW', '0')):
            # split the first load into group pieces so the pipeline starts
            # after the first ~0.7us piece, and slot the tiny const loads in
            # behind piece 0 (they're needed 2+ cycles in)
            first = True
            for p0 in range(0, kw * 256, TGRP * 256):
                p1 = min(kw * 256, p0 + TGRP * 256)
                nc.sync.dma_start(xc_t[:, p0:p1], xc.ap()[w][:, p0:p1])
                if first and const_dmas is not None:
                    for sb_t, dr_t in const_dmas:
                        nc.sync.dma_start(sb_t[:], dr_t.ap())
                    first = False
        else:
            nc.sync.dma_start(xc_t[:, 0 : kw * 256], xc.ap()[w][:, 0 : kw * 256])

        pool_ps = pool_psum.tile([128, 256], F32)
        for g0 in range(0, kw, TGRP):
            gc = min(TGRP, kw - g0)
            # transposes for this group (PE), one fp32 [128,128] per chunk
            tp_ps = tp_pool.tile([128, TGRP * 128], F32)
            for ci in range(gc):
                c = g0 + ci
                xc32 = xc_t[:, c * 256 : (c + 1) * 256].bitcast(F32)
                nc.tensor.transpose(
                    tp_ps[:, ci * 128 : (ci + 1) * 128], xc32, ident32[:])
            # small ops of group g-ST1_LAG (before the copy: Act runs
            # sigmoids ahead of the long copy)
            if len(st1_q) >= ST1_LAG:
                st1_q.pop(0)()
            # copy group PSUM->SBUF on Act (DVE must stay short-latency for
            # the one-hots; a 658ns copy in its queue delays pooled)
            xt_sb = xt_pool.tile([128, TGRP * 128], F32)
            nc.scalar.copy(xt_sb[:, 0 : gc * 128], tp_ps[:, 0 : gc * 128])
            # pooled of group g-ST2_LAG
            if len(st2_q) >= ST2_LAG:
                st2_q.pop(0)()
            s1, s2 = make_stage1(xc_t, xt_sb, w, kw, g0, gc, pool_ps)
            st1_q.append(s1)
            st2_q.append(s2)
    # drain the pipeline, preserving stage1(h) -> stage2(h) order
    while st1_q or st2_q:
        if st1_q and len(st2_q) - len(st1_q) < ST2_LAG - ST1_LAG:
            st1_q.pop(0)()
        if st2_q:
            st2_q.pop(0)()


def _window_plan(batch_idx, nw):
    """Assign the NCORES*nw global windows to (core, slot) so each slot's
    max chunk count over cores is tight, and return (perm, k_list).
    perm[core][slot] = global window id; k_list[slot] = chunk count."""
    bi = np.asarray(batch_idx, dtype=np.int64)
    win_graphs = G // (NCORES * nw)
    bounds = np.searchsorted(bi, np.arange(0, G + 1, win_graphs))
    counts = np.diff(bounds)
    k_all = np.maximum(2, np.ceil(counts / 128).astype(int))
    order = np.argsort(-k_all, kind="stable")
    perm = [[0] * nw for _ in range(NCORES)]
    k_list = []
    for j in range(nw):
        grp = order[j * NCORES : (j + 1) * NCORES]
        k_list.append(int(k_all[grp].max()))
        for c in range(NCORES):
            perm[c][j] = int(grp[c])
    # schedule slots smallest-first (fast pipeline start), then descending
    # (bulk), ending small (short drain)
    slot_order = [nw - 1] + list(range(nw - 1))
    k_list = [k_list[j] for j in slot_order]
    perm = [[perm[c][j] for j in slot_order] for c in range(NCORES)]
    return perm, tuple(k_list), bounds


def _pack_inputs(node_feats, batch_idx, W_attn, b_attn, W_mask, b_mask, nw,
                 perm, k_list, bounds):
    """Pack full inputs into per-core input maps."""
    nf = np.ascontiguousarray(np.asarray(node_feats, dtype=np.float32))
    bi = np.asarray(batch_idx, dtype=np.int64)
    win_graphs = G // (NCORES * nw)
    k = max(k_list)

    np_x = mybir.dt.np(DT_X)
    in_maps = []
    for core in range(NCORES):
        xcb = np.zeros((nw, 128, k * 256), dtype=np_x)
        idxt = np.full((128, nw * k), -1.0, dtype=np.float32)
        for w in range(nw):
            gw = perm[core][w]
            kw = k_list[w]
            s, e = int(bounds[gw]), int(bounds[gw + 1])
            n = e - s
            buf = np.zeros((kw * 128, 256), dtype=np.float32)
            buf[:n] = nf[s:e]
            b3 = buf.reshape(kw, 128, 256)
            # node-major: [p, c*256 + d] = buf[c*128+p, d]
            xcb[w, :, 0 : kw * 256] = (
                b3.transpose(1, 0, 2).reshape(128, kw * 256).astype(np_x))
            # local graph index per node: [p, w*k + c] = idx[c*128+p] - gw*win
            ib = np.full((kw * 128,), -1.0, dtype=np.float32)
            ib[:n] = (bi[s:e] - gw * win_graphs).astype(np.float32)
            idxt[:, w * k : w * k + kw] = ib.reshape(kw, 128).T
        # interleaved weights: wb[j] = [Wa[2j], Wm[2j], Wa[2j+1], Wm[2j+1]]
        wbv = np.zeros((128, 4), dtype=np.float32)
        wa = np.asarray(W_attn, dtype=np.float32).reshape(256)
        wm = np.asarray(W_mask, dtype=np.float32).reshape(256)
        wbv[:, 0] = wa[0::2]
        wbv[:, 1] = wm[0::2]
        wbv[:, 2] = wa[1::2]
        wbv[:, 3] = wm[1::2]
        bbv = np.zeros((128, 2), dtype=np.float32)
        bbv[:, 0] = np.float32(np.asarray(b_attn).reshape(-1)[0])
        bbv[:, 1] = np.float32(np.asarray(b_mask).reshape(-1)[0])
        in_maps.append({
            "xc": xcb, "idxt": idxt,
            "wb": wbv.astype(np_x), "bb": bbv,
        })
    return in_maps


def _compute_k(batch_idx, nw):
    bi = np.asarray(batch_idx, dtype=np.int64)
    win_graphs = G // (NCORES * nw)
    bounds = np.searchsorted(bi, np.arange(0, G + 1, win_graphs))
    counts = np.diff(bounds)
    return max(2, int(np.ceil(counts.max() / 128)))


class _Runner:
    """Compiled SPMD executable with device-resident input support."""

    def __init__(self, nc, n_cores):
        import jax
        from jax.sharding import Mesh, PartitionSpec
        from jax.experimental.shard_map import shard_map
        from concourse.bass2jax import _bass_exec_p, install_neuronx_cc_hook, \
            partition_id_tensor

        install_neuronx_cc_hook()
        in_names, out_names, out_avals, zero_outs = [], [], [], []
        partition_name = (nc.partition_id_tensor.name
                          if nc.partition_id_tensor else None)
        for alloc in nc.m.functions[0].allocations:
            if not isinstance(alloc, mybir.MemoryLocationSet):
                continue
            name = alloc.memorylocations[0].name
            if alloc.kind == "ExternalInput":
                if name != partition_name:
                    in_names.append(name)
            elif alloc.kind == "ExternalOutput":
                shape = tuple(alloc.tensor_shape)
                dtype = mybir.dt.np(alloc.dtype)
                out_names.append(name)
                out_avals.append(jax.core.ShapedArray(shape, dtype))
                zero_outs.append(np.zeros(shape, dtype))
        self.n_params = len(in_names)
        self.in_names = list(in_names)
        self.out_names = out_names
        all_names = in_names + out_names
        if partition_name is not None:
            all_names.append(partition_name)

        def _body(*args):
            operands = list(args)
            if partition_name is not None:
                operands.append(partition_id_tensor())
            outs = _bass_exec_p.bind(
                *operands,
                out_avals=tuple(out_avals),
                in_names=tuple(all_names),
                out_names=tuple(out_names),
                lowering_input_output_aliases=(),
                sim_require_finite=True,
                sim_require_nnan=True,
                nc=nc,
            )
            return tuple(outs)

        devices = jax.devices()[:n_cores]
        self.mesh = Mesh(np.asarray(devices), ("core",))
        n_in = self.n_params + len(out_names)
        self.jitted = jax.jit(
            shard_map(_body, mesh=self.mesh,
                      in_specs=(PartitionSpec("core"),) * n_in,
                      out_specs=(PartitionSpec("core"),) * len(out_names),
                      check_rep=False),
            keep_unused=True,
        )
        self.zero_outs = zero_outs
        self.n_cores = n_cores
        self._jax = jax
        self._P = PartitionSpec

    def put_inputs(self, in_maps):
        """Concatenate per-core inputs and place on device."""
        import jax
        from jax.sharding import NamedSharding
        arrs = []
        for i, name in enumerate(self.in_names):
            cat = np.concatenate([np.asarray(m[name]) for m in in_maps], axis=0)
            arrs.append(cat)
        for z in self.zero_outs:
            arrs.append(np.concatenate([z] * self.n_cores, axis=0))
        sh = NamedSharding(self.mesh, self._P("core"))
        return [jax.device_put(a, sh) for a in arrs]

    def run(self, dev_args):
        return self.jitted(*dev_args)


_runner_cache = {}


def _get_runner(nw, k_list):
    key = (nw, k_list)
    if key not in _runner_cache:
        if key not in _prog_cache:
            _prog_cache[key] = _build_program(nw, k_list)
        _runner_cache[key] = _Runner(_prog_cache[key], NCORES)
    return _runner_cache[key]


def kernel(node_feats, batch_idx, W_attn, b_attn, W_mask, b_mask):
    from concourse.bass_utils import run_bass_kernel_spmd
    nw = NW
    perm, k_list, bounds = _window_plan(batch_idx, nw)
    key = (nw, k_list)
    if key not in _prog_cache:
        _prog_cache[key] = _build_program(nw, k_list)
    nc = _prog_cache[key]
    in_maps = _pack_inputs(node_feats, batch_idx, W_attn, b_attn, W_mask,
                           b_mask, nw, perm, k_list, bounds)
    res = run_bass_kernel_spmd(nc, in_maps, list(range(NCORES)))
    final = np.zeros((G, 256), dtype=np.float32)
    for c in range(NCORES):
        o = np.asarray(res.results[c]["out"], dtype=np.float32)
        for j in range(nw):
            gw = perm[c][j]
            final[gw * WIN : (gw + 1) * WIN] = (
                o[j // 2, :, (j % 2) * 256 : (j % 2 + 1) * 256])
    return final


def _bench_calls(nw, k_list, repeat, in_maps, n_calls=10, warmup=2):
    """Sequential blocking calls of the repeat-looped program; returns list
    of per-call wall times (device execution repeats the computation
    `repeat` times inside one NEFF dispatch)."""
    import time
    key = (nw, k_list, repeat)
    if key not in _runner_cache:
        _runner_cache[key] = _Runner(_build_program(nw, k_list, repeat=repeat),
                                     NCORES)
    runner = _runner_cache[key]
    dev_args = runner.put_inputs(in_maps)
    times = []
    for i in range(warmup + n_calls):
        t0 = time.perf_counter()
        r = runner.run(dev_args)
        np.asarray(r[0])  # force d2h fetch => true completion
        dt = time.perf_counter() - t0
        if i >= warmup:
            times.append(dt)
    return times


def benchmark(node_feats, batch_idx, W_attn, b_attn, W_mask, b_mask,
              r_small=1, r_big=2049):
    """Estimate per-execution device time in ns via repeat-loop differencing."""
    nw = NW
    perm, k_list, bounds = _window_plan(batch_idx, nw)
    in_maps = _pack_inputs(node_feats, batch_idx, W_attn, b_attn, W_mask,
                           b_mask, nw, perm, k_list, bounds)
    t1 = _bench_calls(nw, k_list, r_small, in_maps)
    t2 = _bench_calls(nw, k_list, r_big, in_maps)
    per_exec = (min(t2) - min(t1)) / (r_big - r_small)
    return per_exec * 1e9, min(t1), min(t2), t1, t2


# revision 21
# speedup vs baseline: 1.6304x; 1.0383x over previous
"""Trainium2 Bass kernel for attention pooling (nn_AttentionPooling_26233660244214).

Computation (reference):
    attn = node_feats @ W_attn + b_attn            # [N, 1]
    mask = sigmoid(node_feats @ W_mask + b_mask)   # [N, 1]
    f = attn * mask                                # [N, 1]
    pooled = segment_sum(node_feats * f, batch_idx, 16384)   # [16384, 256]

Strategy: data-parallel over graphs. batch_idx is sorted, so graphs are
contiguous runs of nodes. Each of the 8 cores owns 2048 contiguous graphs,
split into 16 windows of 128 graphs. Host packs each window's nodes into
K fixed-size chunks of 128 nodes (zero-padded) in ONE node-major fp16
layout (single copy of X -> half the HBM traffic of a two-layout scheme).
On device, per chunk of 128 nodes:
  - TensorE transpose (fp32 view = packed fp16 feat pairs) of the chunk
    into PSUM: one [128,128]xf32 is_transpose matmul per chunk. This is an
    exact bit-movement; the fp32 element (X[n,2j], X[n,2j+1]) lands
    transposed so the fp16 view of the output holds X[n,2j] at [j,2n] and
    X[n,2j+1] at [j,2n+1].
  - Act/DVE (alternating per 4-chunk group): copy the transposed group
    PSUM->SBUF ([128, 4*128] fp32, one instruction amortizes access
    latency).
  - TensorE: dots[n,0:2] via 2 matmuls with stride-2 fp16 stationary
    slices (even feats / odd feats) against interleaved [W_attn|W_mask].
  - ScalarE: sig = sigmoid(dots[:,1] + b_mask)
  - VectorE: f = (dots[:,0] + b_attn) * sig
  - VectorE: oh[n, g] = (iota[g] == local_idx[n]) * f[n]         # [128, 128]
  - TensorE: pooled_psum[g, d] += oh.T @ X_chunk  (PSUM accumulate over K)
Transposes are software-pipelined one group ahead of the dots/pool tail so
the PE keeps busy while the copy engine drains the previous group.
Window result [128, 256] is copied PSUM->SBUF on ScalarE and DMA'd out on
the Activation HWDGE ring. Outputs of the 8 cores are concatenated on host
(no cross-core reduction: window boundaries align with graph boundaries).
"""

import os
os.environ.setdefault("JAX_PLATFORMS", "axon,cpu")

import numpy as np
from contextlib import ExitStack

import concourse.bass as bass
import concourse.bacc as bacc
import concourse.tile as tile
from concourse import mybir

N_NODES = 500000
D = 256
G = 16384
NCORES = 8
WIN = 128            # graphs per window
NW = 16              # windows per core
GPC = WIN * NW       # graphs per core
TGRP = int(os.environ.get('K_TGRP', '4'))  # chunks per transpose/copy group

# dtype configuration
DT_X = mybir.dt.float16     # node-major X (pooled rhs; fp32-paired for transpose)
F32 = mybir.dt.float32

XC_BUFS = int(os.environ.get('K_XCBUFS', '4'))  # window-load buffering depth

_prog_cache = {}


def _build_program(nw, k_list, repeat=1):
    """Build the per-core Bass program: window-slot j runs k_list[j] chunks.

    Windows are assigned to (core, slot) by _window_plan so each slot's
    max chunk count over the 8 cores is tight (the SPMD program is shared).
    repeat > 1 wraps the whole computation in a hardware loop executing it
    `repeat` times (for benchmarking)."""
    k_list = list(k_list)
    k = max(k_list)
    nc = bacc.Bacc("TRN2", target_bir_lowering=False, debug=False)

    # xc = node-major fp16 per window: one DMA per window (~370 GB/s).
    xc = nc.dram_tensor("xc", [nw, 128, k * 256], DT_X, kind="ExternalInput")
    idxt = nc.dram_tensor("idxt", [128, nw * k], F32, kind="ExternalInput")
    wb = nc.dram_tensor("wb", [128, 4], DT_X, kind="ExternalInput")
    bb = nc.dram_tensor("bb", [128, 2], F32, kind="ExternalInput")
    ship_ws = _ship_windows(nw)
    xf = (nc.dram_tensor("xf", [len(ship_ws), 128, TGRP * 256], DT_X,
                         kind="ExternalInput") if ship_ws else None)
    # grouped output layout: one 512KB DMA per 4 windows (vs 16 x 128KB);
    # host unscatters (it already permutes windows anyway)
    out = nc.dram_tensor("out", [nw // 2, 128, 2 * 256], F32, kind="ExternalOutput")

    with tile.TileContext(nc) as tc, ExitStack() as ctx:
        const_pool = ctx.enter_context(tc.tile_pool(name="const", bufs=1))
        xc_pool = ctx.enter_context(tc.tile_pool(name="xc", bufs=XC_BUFS))
        tp_pool = ctx.enter_context(tc.tile_pool(name="tp", bufs=int(os.environ.get("K_TPBUFS","3")), space="PSUM"))
        xt_pool = ctx.enter_context(tc.tile_pool(name="xt", bufs=4))
        dots_pool = ctx.enter_context(tc.tile_pool(name="dots", bufs=int(os.environ.get("K_DOTBUFS","2")), space="PSUM"))
        pool_psum = ctx.enter_context(tc.tile_pool(name="pool", bufs=int(os.environ.get("K_PLBUFS","2")), space="PSUM"))
        small_pool = ctx.enter_context(tc.tile_pool(name="small", bufs=12))
        oh_pool = ctx.enter_context(tc.tile_pool(name="oh", bufs=int(os.environ.get("K_OHBUFS","12"))))
        out_pool = ctx.enter_context(tc.tile_pool(name="outp", bufs=int(os.environ.get("K_OUTBUFS","2"))))
        assert nw % 2 == 0

        # constants
        iota_i = const_pool.tile([128, 128], mybir.dt.int32)
        nc.gpsimd.iota(iota_i[:], pattern=[[1, 128]], base=0, channel_multiplier=0)
        iota_f = const_pool.tile([128, 128], DT_X)
        nc.vector.tensor_copy(iota_f[:], iota_i[:])
        # identity (fp32) for TensorE transposes: ident[p, f] = (f == p)
        iota_ci = const_pool.tile([128, 1], mybir.dt.int32)
        nc.gpsimd.iota(iota_ci[:], pattern=[[1, 1]], base=0, channel_multiplier=1)
        iota_cf = const_pool.tile([128, 1], F32)
        nc.vector.tensor_copy(iota_cf[:], iota_ci[:])
        iota_rf = const_pool.tile([128, 128], F32)
        nc.vector.tensor_copy(iota_rf[:], iota_i[:])
        ident32 = const_pool.tile([128, 128], F32)
        nc.vector.tensor_scalar(
            out=ident32[:], in0=iota_rf[:], scalar1=iota_cf[:], scalar2=None,
            op0=mybir.AluOpType.is_equal,
        )
        idx_sb = const_pool.tile([128, nw * k], F32)
        wb_sb = const_pool.tile([128, 4], DT_X)
        bb_sb = const_pool.tile([128, 2], F32)
        const_dmas = [(wb_sb, wb), (bb_sb, bb), (idx_sb, idxt)]

        out_ap = out.ap()

        def body(_iv=None):
            _emit_windows(nc, tc, nw, k, k_list, xc, out_ap, iota_f, ident32,
                          idx_sb, wb_sb, bb_sb, xc_pool, tp_pool, xt_pool,
                          dots_pool, pool_psum, small_pool, oh_pool, out_pool,
                          const_dmas, ship_ws, xf)

        if repeat > 1:
            with tc.For_i(0, repeat, 1):
                body()
        else:
            body()

    nc.compile()
    return nc


def _ship_windows(nw):
    """Windows whose LAST group is shipped pre-transposed (skips the PE
    transpose + PSUM copy for those chunks; costs extra DMA, which has
    headroom). Late windows only: early supply is DMA-starved."""
    spec = os.environ.get('K_SHIPW', '')
    if not spec:
        return []
    return [int(x) for x in spec.split(',') if x != '' and int(x) < nw]


def _emit_windows(nc, tc, nw, k, k_list, xc, out_ap, iota_f, ident32, idx_sb,
                  wb_sb, bb_sb, xc_pool, tp_pool, xt_pool, dots_pool,
                  pool_psum, small_pool, oh_pool, out_pool, const_dmas=None,
                  ship_ws=(), xf=None):
    """Three-stage software pipeline over 4-chunk groups:

      cycle g emits, in this order:
        PE : transposes(g)                  [tp_ps(g) <- X(g)]
        PE : dots(g-1)                      [reads xt_sb(g-1)]
        Act: sig(g-1); DVE: f(g-1), oh(g-1)
        Act/DVE (alternating): copy(g)      [xt_sb(g) <- tp_ps(g)]
        PE : pooled(g-2) (+ window-close output copy/DMA)

    The copy is emitted AFTER the g-1 small ops so Act's queue runs the
    sigmoids before the long copy (no head-of-line blocking), and pooled
    runs a full cycle after oh so the PE never stalls on the one-hot."""
    sigmoid = mybir.ActivationFunctionType.Sigmoid
    alu = mybir.AluOpType

    state = {"out4": None, "copy_tog": -1}

    def make_stage1(xc_t, xt_sb, w, kw, g0, gc, pool_ps):
        oh_list = []

        def stage1():
            xt16 = xt_sb[:].bitcast(DT_X)  # [128, TGRP*256]
            # one dots tile per group: chunk ci owns columns [2ci, 2ci+2)
            dots_g = dots_pool.tile([128, 2 * TGRP], F32)
            for ci in range(gc):
                base = ci * 256
                # dots[n,0:2] = sum_j X[n,2j]*W[2j] + sum_j X[n,2j+1]*W[2j+1]
                nc.tensor.matmul(
                    dots_g[:, 2 * ci : 2 * ci + 2],
                    lhsT=xt16[:, base : base + 256 : 2],
                    rhs=wb_sb[:, 0:2], start=True, stop=False,
                )
                nc.tensor.matmul(
                    dots_g[:, 2 * ci : 2 * ci + 2],
                    lhsT=xt16[:, base + 1 : base + 256 : 2],
                    rhs=wb_sb[:, 2:4], start=False, stop=True,
                )
            # grouped sigmoid / f: one instruction per group
            sig_g = small_pool.tile([128, TGRP], F32, tag="sig")
            nc.scalar.activation(sig_g[:, 0:gc], dots_g[:, 1 : 2 * gc : 2],
                                 sigmoid, bias=bb_sb[:, 1:2], scale=1.0)
            f_g = small_pool.tile([128, TGRP], F32, tag="f")
            nc.vector.scalar_tensor_tensor(
                f_g[:, 0:gc], in0=dots_g[:, 0 : 2 * gc : 2],
                scalar=bb_sb[:, 0:1], in1=sig_g[:, 0:gc],
                op0=alu.add, op1=alu.mult,
            )
            for ci in range(gc):
                c = g0 + ci
                j = w * k + c
                oh = oh_pool.tile([128, 128], DT_X)
                nc.vector.tensor_scalar(
                    out=oh[:], in0=iota_f[:], scalar1=idx_sb[:, j : j + 1],
                    scalar2=f_g[:, ci : ci + 1], op0=alu.is_equal, op1=alu.mult,
                )
                oh_list.append(oh)

        def stage2():
            for ci in range(gc):
                c = g0 + ci
                nc.tensor.matmul(
                    pool_ps[:], lhsT=oh_list[ci][:],
                    rhs=xc_t[:, c * 256 : (c + 1) * 256],
                    start=(c == 0), stop=(c == kw - 1),
                )
            if g0 + gc == kw:
                # window closed: copy result out of PSUM, DMA per 4 windows
                if w % 2 == 0:
                    state["out4"] = out_pool.tile([128, 2 * 256], F32,
                                                  name="out4", tag="out4")
                out4 = state["out4"]
                if os.environ.get('K_OUTDVE', '1') == '1':
                    nc.vector.tensor_copy(
                        out4[:, (w % 2) * 256 : (w % 2 + 1) * 256], pool_ps[:])
                else:
                    nc.scalar.copy(
                        out4[:, (w % 2) * 256 : (w % 2 + 1) * 256], pool_ps[:])
                if w % 2 == 1:
                    if os.environ.get('K_OUTPOOL', '1') == '1':
                        nc.gpsimd.dma_start(out_ap[w // 2], out4[:])
                    else:
                        nc.scalar.dma_start(out_ap[w // 2], out4[:])

        return stage1, stage2

    ST1_LAG = int(os.environ.get('K_ST1LAG', '2'))
    ST2_LAG = int(os.environ.get('K_ST2LAG', '8'))
    st1_q = []        # pending stage1 closures
    st2_q = []        # pending stage2 closures
    for w in range(nw):
        kw = k_list[w]
        xc_t = xc_pool.tile([128, k * 256], DT_X)
        if w <= int(os.environ.get('K_SPLITW', '0')):
            # split the first load into group pieces so the pipeline starts
            # after the first ~0.7us piece, and slot the tiny const loads in
            # behind piece 0 (they're needed 2+ cycles in)
            first = True
            for p0 in range(0, kw * 256, TGRP * 256):
                p1 = min(kw * 256, p0 + TGRP * 256)
                nc.sync.dma_start(xc_t[:, p0:p1], xc.ap()[w][:, p0:p1])
                if first and const_dmas is not None:
                    for sb_t, dr_t in const_dmas:
                        nc.sync.dma_start(sb_t[:], dr_t.ap())
                    first = False
        else:
            nc.sync.dma_start(xc_t[:, 0 : kw * 256], xc.ap()[w][:, 0 : kw * 256])

        pool_ps = pool_psum.tile([128, 256], F32)
        ship_last = w in ship_ws
        for g0 in range(0, kw, TGRP):
            gc = min(TGRP, kw - g0)
            if ship_last and g0 + gc == kw:
                # last group arrives pre-transposed from DRAM
                xt_sb = xt_pool.tile([128, TGRP * 128], F32, name="xt_sb")
                nc.sync.dma_start(
                    xt_sb[:].bitcast(DT_X)[:, 0 : gc * 256],
                    xf.ap()[list(ship_ws).index(w)][:, 0 : gc * 256])
                if len(st1_q) >= ST1_LAG:
                    st1_q.pop(0)()
                if len(st2_q) >= ST2_LAG:
                    st2_q.pop(0)()
                s1, s2 = make_stage1(xc_t, xt_sb, w, kw, g0, gc, pool_ps)
                st1_q.append(s1)
                st2_q.append(s2)
                continue
            # transposes for this group (PE), one fp32 [128,128] per chunk
            tp_ps = tp_pool.tile([128, TGRP * 128], F32)
            for ci in range(gc):
                c = g0 + ci
                xc32 = xc_t[:, c * 256 : (c + 1) * 256].bitcast(F32)
                nc.tensor.transpose(
                    tp_ps[:, ci * 128 : (ci + 1) * 128], xc32, ident32[:])
            # small ops of group g-ST1_LAG (before the copy: Act runs
            # sigmoids ahead of the long copy)
            if len(st1_q) >= ST1_LAG:
                st1_q.pop(0)()
            # copy group PSUM->SBUF on Act (DVE must stay short-latency for
            # the one-hots; a 658ns copy in its queue delays pooled)
            xt_sb = xt_pool.tile([128, TGRP * 128], F32)
            cpdve = int(os.environ.get('K_CPDVE', '0'))
            state["copy_tog"] += 1
            if cpdve and state["copy_tog"] % cpdve == 0:
                nc.vector.tensor_copy(xt_sb[:, 0 : gc * 128],
                                      tp_ps[:, 0 : gc * 128])
            else:
                nc.scalar.copy(xt_sb[:, 0 : gc * 128], tp_ps[:, 0 : gc * 128])
            # pooled of group g-ST2_LAG
            if len(st2_q) >= ST2_LAG:
                st2_q.pop(0)()
            s1, s2 = make_stage1(xc_t, xt_sb, w, kw, g0, gc, pool_ps)
            st1_q.append(s1)
            st2_q.append(s2)
    # drain the pipeline, preserving stage1(h) -> stage2(h) order
    while st1_q or st2_q:
        if st1_q and len(st2_q) - len(st1_q) < ST2_LAG - ST1_LAG:
            st1_q.pop(0)()
        if st2_q:
            st2_q.pop(0)()


def _window_plan(batch_idx, nw):
    """Assign the NCORES*nw global windows to (core, slot) so each slot's
    max chunk count over cores is tight, and return (perm, k_list).
    perm[core][slot] = global window id; k_list[slot] = chunk count."""
    bi = np.asarray(batch_idx, dtype=np.int64)
    win_graphs = G // (NCORES * nw)
    bounds = np.searchsorted(bi, np.arange(0, G + 1, win_graphs))
    counts = np.diff(bounds)
    k_all = np.maximum(2, np.ceil(counts / 128).astype(int))
    order = np.argsort(-k_all, kind="stable")
    perm = [[0] * nw for _ in range(NCORES)]
    k_list = []
    for j in range(nw):
        grp = order[j * NCORES : (j + 1) * NCORES]
        k_list.append(int(k_all[grp].max()))
        for c in range(NCORES):
            perm[c][j] = int(grp[c])
    # schedule slots smallest-first (fast pipeline start), then descending
    # (bulk), ending small (short drain)
    slot_order = [nw - 1] + list(range(nw - 1))
    k_list = [k_list[j] for j in slot_order]
    perm = [[perm[c][j] for j in slot_order] for c in range(NCORES)]
    return perm, tuple(k_list), bounds


def _pack_inputs(node_feats, batch_idx, W_attn, b_attn, W_mask, b_mask, nw,
                 perm, k_list, bounds):
    """Pack full inputs into per-core input maps."""
    nf = np.ascontiguousarray(np.asarray(node_feats, dtype=np.float32))
    bi = np.asarray(batch_idx, dtype=np.int64)
    win_graphs = G // (NCORES * nw)
    k = max(k_list)
    ship_ws = _ship_windows(nw)

    np_x = mybir.dt.np(DT_X)
    in_maps = []
    for core in range(NCORES):
        xcb = np.zeros((nw, 128, k * 256), dtype=np_x)
        xfb = (np.zeros((len(ship_ws), 128, TGRP * 256), dtype=np_x)
               if ship_ws else None)
        idxt = np.full((128, nw * k), -1.0, dtype=np.float32)
        for w in range(nw):
            gw = perm[core][w]
            kw = k_list[w]
            s, e = int(bounds[gw]), int(bounds[gw + 1])
            n = e - s
            buf = np.zeros((kw * 128, 256), dtype=np.float32)
            buf[:n] = nf[s:e]
            b3 = buf.reshape(kw, 128, 256)
            # node-major: [p, c*256 + d] = buf[c*128+p, d]
            xcb[w, :, 0 : kw * 256] = (
                b3.transpose(1, 0, 2).reshape(128, kw * 256).astype(np_x))
            if w in ship_ws:
                # last group pre-transposed (post-PE-transpose layout):
                # xf[j, ci*256 + 2n + r] = chunk[g0+ci][n, 2j + r]
                g0 = ((kw - 1) // TGRP) * TGRP
                gcs = kw - g0
                t = b3[g0:kw].reshape(gcs, 128, 128, 2)   # [ci, n, j, r]
                xfb[list(ship_ws).index(w), :, 0 : gcs * 256] = (
                    t.transpose(0, 2, 1, 3).reshape(gcs, 128, 256)
                     .transpose(1, 0, 2).reshape(128, gcs * 256).astype(np_x))
            # local graph index per node: [p, w*k + c] = idx[c*128+p] - gw*win
            ib = np.full((kw * 128,), -1.0, dtype=np.float32)
            ib[:n] = (bi[s:e] - gw * win_graphs).astype(np.float32)
            idxt[:, w * k : w * k + kw] = ib.reshape(kw, 128).T
        # interleaved weights: wb[j] = [Wa[2j], Wm[2j], Wa[2j+1], Wm[2j+1]]
        wbv = np.zeros((128, 4), dtype=np.float32)
        wa = np.asarray(W_attn, dtype=np.float32).reshape(256)
        wm = np.asarray(W_mask, dtype=np.float32).reshape(256)
        wbv[:, 0] = wa[0::2]
        wbv[:, 1] = wm[0::2]
        wbv[:, 2] = wa[1::2]
        wbv[:, 3] = wm[1::2]
        bbv = np.zeros((128, 2), dtype=np.float32)
        bbv[:, 0] = np.float32(np.asarray(b_attn).reshape(-1)[0])
        bbv[:, 1] = np.float32(np.asarray(b_mask).reshape(-1)[0])
        m = {"xc": xcb, "idxt": idxt, "wb": wbv.astype(np_x), "bb": bbv}
        if xfb is not None:
            m["xf"] = xfb
        in_maps.append(m)
    return in_maps


def _compute_k(batch_idx, nw):
    bi = np.asarray(batch_idx, dtype=np.int64)
    win_graphs = G // (NCORES * nw)
    bounds = np.searchsorted(bi, np.arange(0, G + 1, win_graphs))
    counts = np.diff(bounds)
    return max(2, int(np.ceil(counts.max() / 128)))


class _Runner:
    """Compiled SPMD executable with device-resident input support."""

    def __init__(self, nc, n_cores):
        import jax
        from jax.sharding import Mesh, PartitionSpec
        from jax.experimental.shard_map import shard_map
        from concourse.bass2jax import _bass_exec_p, install_neuronx_cc_hook, \
            partition_id_tensor

        install_neuronx_cc_hook()
        in_names, out_names, out_avals, zero_outs = [], [], [], []
        partition_name = (nc.partition_id_tensor.name
                          if nc.partition_id_tensor else None)
        for alloc in nc.m.functions[0].allocations:
            if not isinstance(alloc, mybir.MemoryLocationSet):
                continue
            name = alloc.memorylocations[0].name
            if alloc.kind == "ExternalInput":
                if name != partition_name:
                    in_names.append(name)
            elif alloc.kind == "ExternalOutput":
                shape = tuple(alloc.tensor_shape)
                dtype = mybir.dt.np(alloc.dtype)
                out_names.append(name)
                out_avals.append(jax.core.ShapedArray(shape, dtype))
                zero_outs.append(np.zeros(shape, dtype))
        self.n_params = len(in_names)
        self.in_names = list(in_names)
        self.out_names = out_names
        all_names = in_names + out_names
        if partition_name is not None:
            all_names.append(partition_name)

        def _body(*args):
            operands = list(args)
            if partition_name is not None:
                operands.append(partition_id_tensor())
            outs = _bass_exec_p.bind(
                *operands,
                out_avals=tuple(out_avals),
                in_names=tuple(all_names),
                out_names=tuple(out_names),
                lowering_input_output_aliases=(),
                sim_require_finite=True,
                sim_require_nnan=True,
                nc=nc,
            )
            return tuple(outs)

        devices = jax.devices()[:n_cores]
        self.mesh = Mesh(np.asarray(devices), ("core",))
        n_in = self.n_params + len(out_names)
        self.jitted = jax.jit(
            shard_map(_body, mesh=self.mesh,
                      in_specs=(PartitionSpec("core"),) * n_in,
                      out_specs=(PartitionSpec("core"),) * len(out_names),
                      check_rep=False),
            keep_unused=True,
        )
        self.zero_outs = zero_outs
        self.n_cores = n_cores
        self._jax = jax
        self._P = PartitionSpec

    def put_inputs(self, in_maps):
        """Concatenate per-core inputs and place on device."""
        import jax
        from jax.sharding import NamedSharding
        arrs = []
        for i, name in enumerate(self.in_names):
            cat = np.concatenate([np.asarray(m[name]) for m in in_maps], axis=0)
            arrs.append(cat)
        for z in self.zero_outs:
            arrs.append(np.concatenate([z] * self.n_cores, axis=0))
        sh = NamedSharding(self.mesh, self._P("core"))
        return [jax.device_put(a, sh) for a in arrs]

    def run(self, dev_args):
        return self.jitted(*dev_args)


_runner_cache = {}


def _get_runner(nw, k_list):
    key = (nw, k_list)
    if key not in _runner_cache:
        if key not in _prog_cache:
            _prog_cache[key] = _build_program(nw, k_list)
        _runner_cache[key] = _Runner(_prog_cache[key], NCORES)
    return _runner_cache[key]


def kernel(node_feats, batch_idx, W_attn, b_attn, W_mask, b_mask):
    from concourse.bass_utils import run_bass_kernel_spmd
    nw = NW
    perm, k_list, bounds = _window_plan(batch_idx, nw)
    key = (nw, k_list)
    if key not in _prog_cache:
        _prog_cache[key] = _build_program(nw, k_list)
    nc = _prog_cache[key]
    in_maps = _pack_inputs(node_feats, batch_idx, W_attn, b_attn, W_mask,
                           b_mask, nw, perm, k_list, bounds)
    res = run_bass_kernel_spmd(nc, in_maps, list(range(NCORES)))
    final = np.zeros((G, 256), dtype=np.float32)
    for c in range(NCORES):
        o = np.asarray(res.results[c]["out"], dtype=np.float32)
        for j in range(nw):
            gw = perm[c][j]
            final[gw * WIN : (gw + 1) * WIN] = (
                o[j // 2, :, (j % 2) * 256 : (j % 2 + 1) * 256])
    return final


def _bench_calls(nw, k_list, repeat, in_maps, n_calls=10, warmup=2):
    """Sequential blocking calls of the repeat-looped program; returns list
    of per-call wall times (device execution repeats the computation
    `repeat` times inside one NEFF dispatch)."""
    import time
    key = (nw, k_list, repeat)
    if key not in _runner_cache:
        _runner_cache[key] = _Runner(_build_program(nw, k_list, repeat=repeat),
                                     NCORES)
    runner = _runner_cache[key]
    dev_args = runner.put_inputs(in_maps)
    times = []
    for i in range(warmup + n_calls):
        t0 = time.perf_counter()
        r = runner.run(dev_args)
        np.asarray(r[0])  # force d2h fetch => true completion
        dt = time.perf_counter() - t0
        if i >= warmup:
            times.append(dt)
    return times


def benchmark(node_feats, batch_idx, W_attn, b_attn, W_mask, b_mask,
              r_small=1, r_big=2049):
    """Estimate per-execution device time in ns via repeat-loop differencing."""
    nw = NW
    perm, k_list, bounds = _window_plan(batch_idx, nw)
    in_maps = _pack_inputs(node_feats, batch_idx, W_attn, b_attn, W_mask,
                           b_mask, nw, perm, k_list, bounds)
    t1 = _bench_calls(nw, k_list, r_small, in_maps)
    t2 = _bench_calls(nw, k_list, r_big, in_maps)
    per_exec = (min(t2) - min(t1)) / (r_big - r_small)
    return per_exec * 1e9, min(t1), min(t2), t1, t2
